# revision 1
# baseline (speedup 1.0000x reference)
"""AttentionAggregator kernel for 8 TRN2 NeuronCores.

Reference computation (per node i over M neighbors j):
    s_self  = self_feats @ a[:D]            # [N]
    s_neigh = features_neighs @ a[D:]       # [M]
    scores  = leaky_relu(s_self[:,None] + s_neigh[None,:], 0.2)
    attn    = softmax(where(mask, scores, -1e30), axis=1); attn = where(mask, attn, 0)
    out     = attn @ features_neighs        # [N, D]

Key identity used on device: with x = s_self[i] + s_neigh[j],
    exp(leaky_relu(x, 0.2)) = max(exp(x), exp(0.2 x)) = max(u_i*v_j, uh_i*vh_j)
where u = exp(s_self), uh = exp(0.2 s_self), v = exp(s_neigh), vh = exp(0.2 s_neigh).
Softmax max-subtraction is skipped (scores are O(10), exp is safe in fp32/bf16),
and masked entries are exactly zero after multiplying by the 0/1 mask, so
    out[i] = (p @ F)[i] / rowsum(p)[i],   p = mask * max(u v, uh vh).

Per node tile [128 x 4096] (all bf16 elementwise):
    DMA : mask tile loaded via SWDGE with inline int32 -> bf16 cast
    ACT : A = v_bcast * u[i]                  (Copy with per-partition scale)
    DVE : B = vh_bcast * uh[i]                (tensor_scalar, 4x mode)
    DVE : C = max(B, A)                       (tensor_tensor, 2x mode)
    DVE : p = C * mask                        (tensor_tensor, 2x mode)
    DMA : pT = xbar blocked transpose of p    (one InstDmaTransposeAnt)
    PE  : psum[128,129] += pT_c^T @ [F_c | 1] (32 accumulating bf16 matmuls;
                                               ones column yields rowsum)
    DVE : out = psum[:, :128] * (1/rowsum)

Sharding: rows of self_feats / neigh_matrix split across 8 cores (2048 rows
each); features_neighs and `a` replicated. No collectives.
"""

import numpy as np
from contextlib import ExitStack

N, M, D = 16384, 4096, 128
NCORES = 8
NLOC = N // NCORES          # 2048 nodes per core
P = 128                     # partitions
NT = NLOC // P              # 16 node tiles per core
NC_J = M // P               # 32 neighbor chunks

_BUILT = {}


def _build_nc(n_loc=NLOC, m=M, d=D, reps=1, fuse_mask_dma=False,
              mm_fstat=False):
    import concourse.bass as bass
    import concourse.bacc as bacc
    import concourse.tile as tile
    from concourse import mybir, masks

    f32 = mybir.dt.float32
    bf16 = mybir.dt.bfloat16
    i32 = mybir.dt.int32
    Op = mybir.AluOpType
    AF = mybir.ActivationFunctionType

    nt = n_loc // P
    nj = m // P

    nc = bacc.Bacc("TRN2", target_bir_lowering=False, debug=False,
                   num_devices=NCORES)

    self_d = nc.dram_tensor("self_feats", [n_loc, d], f32, kind="ExternalInput")
    f_d = nc.dram_tensor("features_neighs", [m, d], f32, kind="ExternalInput")
    m_d = nc.dram_tensor("neigh_matrix", [n_loc, m], i32, kind="ExternalInput")
    a_d = nc.dram_tensor("a", [2 * d, 1], f32, kind="ExternalInput")
    out_d = nc.dram_tensor("out", [n_loc, d], f32, kind="ExternalOutput")

    with tile.TileContext(nc) as tc, ExitStack() as ctx:
        const = ctx.enter_context(tc.tile_pool(name="const", bufs=1))
        maskp = ctx.enter_context(tc.tile_pool(name="maskp", bufs=14))
        pre_ctx = ExitStack()
        pre = pre_ctx.enter_context(tc.tile_pool(name="pre", bufs=4))
        stage = pre_ctx.enter_context(tc.tile_pool(name="stage", bufs=1))
        psum_pre = pre_ctx.enter_context(
            tc.tile_pool(name="psum_pre", bufs=2, space="PSUM"))
        psum_row = pre_ctx.enter_context(
            tc.tile_pool(name="psum_row", bufs=4, space="PSUM"))

        # F quarter-slice loads go on the otherwise-idle HWDGE ring (the
        # SWDGE ring belongs to the mask stream); each slice is cast, dotted,
        # exponentiated, and broadcast while later slices are still in
        # flight, so broadcast work overlaps the F stream.
        f_sb = const.tile([P, nj, P + 1], bf16)
        f_stage = stage.tile([P, nj, P], f32, tag="fstage")
        NQ = 4 if nj % 4 == 0 else 1
        qs = nj // NQ
        f_src = f_d.ap().rearrange("(c q) d -> q c d", q=P)
        for q in range(NQ):
            nc.sync.dma_start(f_stage[:, q * qs:(q + 1) * qs, :],
                              f_src[:, q * qs:(q + 1) * qs, :])

        # prefetch the first mask tiles (casting int32 -> bf16) so the HBM
        # stream saturates from t=0, overlapping the whole precompute
        mask_tiles = {}
        if not fuse_mask_dma:
            for t in range(min(14, nt * reps)):
                mt = maskp.tile([P, m], bf16, tag="mask")
                nc.gpsimd.dma_start(mt[:], m_d[(t % nt) * P:(t % nt + 1) * P, :])
                mask_tiles[t] = mt

        # ---------------- precompute ----------------
        ident = const.tile([P, P], f32)
        masks.make_identity(nc, ident[:])

        ones1 = const.tile([1, P], f32)
        nc.vector.memset(ones1[:], 1.0)

        # selector matrices: sel4[par, k, :] = 1 where par == k, else 0
        # (rows of the 4x4 identity, broadcast along the free dim)
        sel4 = const.tile([4, 4, P], f32)
        nc.vector.tensor_copy(
            sel4[:], ident[0:4, 0:4].unsqueeze(2).to_broadcast([4, 4, P]))

        # a as two single-partition rows: a_self, a_neigh
        a_self_row = const.tile([1, d], f32)
        a_neigh_row = const.tile([1, d], f32)
        a_flat = a_d.ap().rearrange("(one dd) o2 -> one (dd o2)", one=1)
        nc.scalar.dma_start(a_self_row[:], a_flat[:, 0:d])
        nc.scalar.dma_start(a_neigh_row[:], a_flat[:, d:2 * d])

        # broadcast a rows across all 128 partitions via PE outer product
        a_self_b = const.tile([P, d], f32)
        a_neigh_b = const.tile([P, d], f32)
        for dst, row in ((a_self_b, a_self_row), (a_neigh_b, a_neigh_row)):
            ps = psum_pre.tile([P, d], f32, tag="ps_bc")
            nc.tensor.matmul(ps[:], ones1[:], row[:])
            nc.vector.tensor_copy(dst[:], ps[:])

        a_neigh_bb = const.tile([P, d], bf16)
        nc.vector.tensor_copy(a_neigh_bb[:], a_neigh_b[:])

        s_self_c = const.tile([P, nt], f32)     # [q, t] -> s_self[t*128+q]
        s_neigh_c = const.tile([P, nj], f32)    # [q, c] -> s_neigh[c*128+q]

        # self-side dot products, batched: broadcast-multiply + 3D reduce
        self_sb = stage.tile([P, nt, d], f32, tag="selfstage")
        nc.scalar.dma_start(
            self_sb[:], self_d.ap().rearrange("(t q) d -> q t d", q=P))
        prod_s = stage.tile([P, nt, d], f32, tag="prod_s")
        nc.vector.tensor_tensor(
            prod_s[:], self_sb[:],
            a_self_b[:].unsqueeze(1).to_broadcast([P, nt, d]), Op.mult)
        nc.vector.tensor_reduce(s_self_c[:], prod_s[:],
                                mybir.AxisListType.X, Op.add)
        u_c = const.tile([P, nt], f32)
        uh_c = const.tile([P, nt], f32)
        nc.scalar.activation(u_c[:], s_self_c[:], AF.Exp)
        nc.scalar.activation(uh_c[:], s_self_c[:], AF.Exp, scale=0.2)

        # neighbor side per F quarter: cast -> dot -> exp -> broadcast
        v_c = const.tile([P, nj], f32)
        vh_c = const.tile([P, nj], f32)
        vb = const.tile([P, nj, P], bf16)
        vhb = const.tile([P, nj, P], bf16)
        prod_n = stage.tile([P, nj, d], bf16, tag="prod_n")
        for q in range(NQ):
            lo_q, hi_q = q * qs, (q + 1) * qs
            nc.vector.tensor_copy(f_sb[:, lo_q:hi_q, 0:P],
                                  f_stage[:, lo_q:hi_q, :])
            nc.vector.tensor_tensor(
                prod_n[:, lo_q:hi_q, :], f_sb[:, lo_q:hi_q, 0:P],
                a_neigh_bb[:].unsqueeze(1).to_broadcast([P, qs, d]), Op.mult)
            nc.vector.tensor_reduce(s_neigh_c[:, lo_q:hi_q],
                                    prod_n[:, lo_q:hi_q, :],
                                    mybir.AxisListType.X, Op.add)
            nc.scalar.activation(v_c[:, lo_q:hi_q], s_neigh_c[:, lo_q:hi_q],
                                 AF.Exp)
            nc.scalar.activation(vh_c[:, lo_q:hi_q], s_neigh_c[:, lo_q:hi_q],
                                 AF.Exp, scale=0.2)
            for src, dst in ((v_c, vb), (vh_c, vhb)):
                for c0 in range(lo_q, hi_q, 4):
                    nb = min(c0 + 4, hi_q) - c0
                    bank = psum_pre.tile([P, 4 * P], f32, tag="ps_bank")
                    # transpose 4 columns at once -> [4, 128] psum rows
                    pst4 = psum_row.tile([4, P], f32, tag="ps_row")
                    nc.tensor.transpose(pst4[0:nb, :], src[:, c0:c0 + nb],
                                        ident[:])
                    rows4 = pre.tile([4, P], f32, tag="pre_row")
                    nc.vector.tensor_copy(rows4[0:nb, :], pst4[0:nb, :])
                    for k in range(nb):
                        # selector E_k (ones in partition-row k) extracts and
                        # broadcasts row k across all 128 partitions
                        nc.tensor.matmul(bank[:, k * P:(k + 1) * P],
                                         sel4[0:nb, k, :], rows4[0:nb, :])
                    nc.scalar.copy(
                        dst[:, c0:c0 + nb, :].rearrange("p c q -> p (c q)"),
                        bank[:, 0:nb * P])

        nc.gpsimd.memset(f_sb[:].rearrange("p c q -> p (c q)")
                         [:, P::P + 1], 1.0)  # ones column per chunk

        vb_flat = vb[:].rearrange("p c q -> p (c q)")
        vhb_flat = vhb[:].rearrange("p c q -> p (c q)")

        pre_ctx.close()  # release precompute SBUF/PSUM pools

        G = (4 if nt % 4 == 0 else 2 if nt % 2 == 0 else 1) if mm_fstat else 1
        worka = ctx.enter_context(
            tc.tile_pool(name="worka", bufs=2))
        workc = ctx.enter_context(
            tc.tile_pool(name="workc", bufs=2))
        workp = ctx.enter_context(
            tc.tile_pool(name="workp", bufs=2))
        ptp = ctx.enter_context(
            tc.tile_pool(name="ptp", bufs=2))
        psum_mm = ctx.enter_context(
            tc.tile_pool(name="psum_mm", bufs=4 if mm_fstat else 6,
                         space="PSUM"))
        outp = ctx.enter_context(tc.tile_pool(name="outp", bufs=3))
        small = ctx.enter_context(tc.tile_pool(name="small", bufs=8))

        def elementwise(rep, t, accum_rs=None):
            """Produce p_t (masked exp scores); optionally row-sum into
            accum_rs via the fused (1x-rate) scalar_tensor_tensor."""
            a_t = worka.tile([P, m], bf16, tag="a")
            nc.scalar.mul(a_t[:], vb_flat, u_c[:, t:t + 1])
            c_t = workc.tile([P, m], bf16, tag="c")
            nc.vector.tensor_scalar_mul(c_t[:], vhb_flat, uh_c[:, t:t + 1])
            nc.vector.tensor_tensor(c_t[:], c_t[:], a_t[:], Op.max)
            gi = rep * nt + t
            if gi in mask_tiles:
                mask_t = mask_tiles.pop(gi)
            else:
                mask_t = maskp.tile([P, m], bf16, tag="mask")
                nc.gpsimd.dma_start(mask_t[:], m_d[t * P:(t + 1) * P, :])
            p_t = workp.tile([P, m], bf16, tag="p")
            if accum_rs is not None:
                nc.vector.scalar_tensor_tensor(
                    p_t[:], c_t[:], 1.0, mask_t[:], Op.mult, Op.mult,
                    accum_out=accum_rs)
            else:
                nc.vector.tensor_tensor(p_t[:], c_t[:], mask_t[:], Op.mult)
            return p_t

        # ---------------- main loop over node tiles ----------------
        if not mm_fstat:
            for rep in range(reps):
                for t in range(nt):
                    p_t = elementwise(rep, t)
                    # blocked transpose: pT[q, c, r] = p[r, c*128+q]
                    pt_t = ptp.tile([P, nj, P], bf16)
                    nc.sync.dma_start(pt_t[:], p_t[:], transpose=True)
                    # psum[128, 129] += pT_c^T @ [F_c | 1]
                    acc = psum_mm.tile([P, d + 1], f32)
                    for c in range(nj):
                        nc.tensor.matmul(acc[:], pt_t[:, c, :], f_sb[:, c, :],
                                         start=(c == 0), stop=(c == nj - 1))
                    rec_t = small.tile([P, 1], f32, tag="rec")
                    nc.vector.reciprocal(rec_t[:], acc[:, d:d + 1])
                    o_t = outp.tile([P, d], f32)
                    nc.vector.tensor_scalar_mul(o_t[:], acc[:, 0:d], rec_t[:])
                    nc.sync.dma_start(out_d[t * P:(t + 1) * P, :], o_t[:])
        else:
            # F-stationary: per group of G node tiles, 32 weight loads and 32
            # wide matmuls (rhs = G tiles' pT chunks side by side); rowsums on
            # GPSIMD; output comes out transposed and is xbar-transposed back
            # in bf16 before the reciprocal scale.
            assert nt % G == 0
            for rep in range(reps):
                for g in range(nt // G):
                    recs = []
                    ptg = ptp.tile([P, nj, G, P], bf16, tag="ptg")
                    for ti in range(G):
                        t = g * G + ti
                        rs_t = small.tile([P, 1], f32, tag="rs")
                        p_t = elementwise(rep, t, accum_rs=rs_t[:])
                        rec_t = small.tile([P, 1], f32, tag="rec")
                        nc.vector.reciprocal(rec_t[:], rs_t[:])
                        recs.append(rec_t)
                        nc.sync.dma_start(ptg[:, :, ti, :], p_t[:],
                                          transpose=True)
                    accT = psum_mm.tile([P, G * P], f32)
                    for c in range(nj):
                        nc.tensor.matmul(
                            accT[:], f_sb[:, c, 0:P],
                            ptg[:, c, :, :].rearrange("p g q -> p (g q)"),
                            start=(c == 0), stop=(c == nj - 1))
                    outT = outp.tile([P, G * P], bf16, tag="outT")
                    nc.scalar.copy(outT[:], accT[:])
                    o4 = outp.tile([P, G, P], bf16, tag="o4")
                    nc.sync.dma_start(o4[:], outT[:], transpose=True)
                    for ti in range(G):
                        t = g * G + ti
                        o_t = outp.tile([P, d], f32, tag="of")
                        nc.vector.tensor_scalar_mul(o_t[:], o4[:, ti, :],
                                                    recs[ti][:])
                        nc.sync.dma_start(out_d[t * P:(t + 1) * P, :], o_t[:])

    nc.compile()
    return nc


def _get_nc(key=(NLOC, M, D)):
    if key not in _BUILT:
        _BUILT[key] = _build_nc(*key)
    return _BUILT[key]


def kernel(self_feats, features_neighs, neigh_matrix, a):
    from concourse.bass_utils import run_bass_kernel_spmd

    self_feats = np.ascontiguousarray(self_feats, dtype=np.float32)
    features_neighs = np.ascontiguousarray(features_neighs, dtype=np.float32)
    neigh_matrix = np.ascontiguousarray(neigh_matrix, dtype=np.int32)
    a = np.ascontiguousarray(a, dtype=np.float32)

    nc = _get_nc()
    in_maps = []
    for c in range(NCORES):
        sl = slice(c * NLOC, (c + 1) * NLOC)
        in_maps.append({
            "self_feats": self_feats[sl],
            "features_neighs": features_neighs,
            "neigh_matrix": neigh_matrix[sl],
            "a": a,
        })
    res = run_bass_kernel_spmd(nc, in_maps, core_ids=list(range(NCORES)))
    out = np.concatenate([np.asarray(res.results[c]["out"])
                          for c in range(NCORES)], axis=0)
    return out.astype(np.float32)

